# revision 13
# baseline (speedup 1.0000x reference)
"""Trainium2 Bass kernel for CommutatorConv2d.

Math: with lambda_c=0, lambda_a=1 the reference is a conv2d with effective
kernel  w_eff[o,i,r,s] = krow[o,i,s] + kcol[o,i,r]  (krow = sum_r w, kcol =
sum_s w).  The 9-tap conv factors into 1D convs over box-summed inputs, and
because sum_s xv_s == sum_r xh_r == P (the 3x3 patch sum), the 6 natural
contraction slices collapse to 5:

  y[o,h,w] = (krow0-krow2)[o,i] * xv[i, h, w]
           + (krow1-krow2)[o,i] * xv[i, h, w+1]
           + (kcol0-kcol2)[o,i] * xh[i, h,   w]
           + (kcol1-kcol2)[o,i] * xh[i, h+1, w]
           + (krow2+kcol2)[o,i] * P [i, h, w]      + bias[o]

where xv = vertical 3-tap sum of zero-padded x, xh = horizontal 3-tap sum,
P = horizontal 3-tap sum of xv.  Per output tile that is 5 accumulating
matmuls (contraction 128 each) instead of 9 for direct conv / 6 for the
two-1D-conv form.  Weight differences are computed host-side for free.

Schedule: tile-major (a tile's 5 matmuls then its PSUM drain) so drains and
output stores spread across the whole run and the kernel tail is one tile
deep.  Image 0 arrives in 4 row-chunks with box-sums and matmuls emitted
per chunk: the PE stream starts early and never gaps, which also keeps the
HAM clock-gate from re-throttling mid-run.

Output is stored bf16 on device (halves store DMA) and upcast on host.

Sharding: data-parallel over batch; 4 images per core on 8 cores.
"""

import numpy as np
import ml_dtypes

import concourse.bass as bass
import concourse.bacc as bacc
import concourse.mybir as mybir
import concourse.tile as tile
from concourse.bass_utils import run_bass_kernel_spmd

B, CI, CO, H, W = 32, 128, 256, 56, 56
NCORES = 8
BPC = B // NCORES          # images per core
HP, WP = H + 2, W + 2      # padded spatial dims
NPIX = H * W               # 3136
ROWT = 8                   # output rows per matmul tile
NT = H // ROWT             # 7 pixel tiles per image
NTILE = ROWT * W           # 448 columns per matmul
NSL = 5                    # contraction slices

F32 = mybir.dt.float32
BF16 = mybir.dt.bfloat16


def build_nc():
    nc = bacc.Bacc(None, enable_partition_id=False)
    xin = nc.declare_dram_parameter("xp", [BPC, CI, HP, WP], BF16, isOutput=False)
    wk = nc.declare_dram_parameter("klhs", [CI, NSL, CO], BF16, isOutput=False)
    bb = nc.declare_dram_parameter("bias2", [CI, 2], F32, isOutput=False)
    y = nc.declare_dram_parameter("y", [BPC, CO, H, W], BF16, isOutput=True)

    xflat = xin.rearrange("b c h w -> b c (h w)")
    yflat = y.rearrange("b o h w -> b o (h w)")
    NPAD = HP * WP           # 3364
    NV = H * WP              # 3248 (rows 0..55 of padded, all 58 cols)

    with tile.TileContext(nc) as tc:
        with (
            tc.tile_pool(name="const", bufs=1) as cpool,
            tc.tile_pool(name="xp", bufs=BPC) as xpool,
            tc.tile_pool(name="xv", bufs=2) as vpool,
            tc.tile_pool(name="xh", bufs=2) as hpool,
            tc.tile_pool(name="pp", bufs=2) as ppool,
            tc.tile_pool(name="yo", bufs=3) as ypool,
            tc.tile_pool(name="ps", bufs=7, space="PSUM") as pspool,
        ):
            klhs_sb = cpool.tile([CI, NSL * CO], BF16)
            bias_sb = cpool.tile([CI, 2], F32)
            kl3 = klhs_sb.rearrange("i (t o) -> i t o", o=CO)

            # PE warmup: a burst of matmuls on zeros bridges the PE from
            # engine-open until the first real matmul's data has landed
            # (DMA + box-sum chain, ~5us), so the HAM clock-gate reaches
            # 2.4 GHz before the real stream begins and never re-throttles.
            warm = cpool.tile([128, NTILE], BF16)
            nc.vector.memset(warm[:], 0.0)
            warm_ps = pspool.tile([128, NTILE], F32, bufs=1, tag="warm")
            for _ in range(11):
                nc.tensor.matmul(
                    warm_ps[:], warm[:, 0:128], warm[:], start=True, stop=True
                )
            warm_out = cpool.tile([128, 32], F32)
            nc.scalar.activation(
                warm_out[:], warm_ps[:, 0:32], mybir.ActivationFunctionType.Copy
            )

            # All input DMAs are issued before any compute/store is emitted:
            # the sync queue issues strictly in program order, so a load
            # emitted after a store would wait for that store's (compute-
            # gated) semaphore — serializing input prefetch behind compute.
            ROW_CHUNKS0 = [12, 30, 44, HP]
            xp_sbs = []
            for b in range(BPC):
                xp_sb = xpool.tile([CI, NPAD], BF16, name=f"xp_{b}")
                xp_sbs.append(xp_sb)
            # chunk 1 of image 0 leads (it gates the box-sum chain, which is
            # longer than the weights->first-matmul path), then weights,
            # then the rest of the input.
            xp3d0 = xflat[0].rearrange("i (h w) -> i h w", w=WP)
            xps30 = xp_sbs[0].rearrange("i (h w) -> i h w", w=WP)
            nc.sync.dma_start(
                out=xps30[:, 0 : ROW_CHUNKS0[0], :],
                in_=xp3d0[:, 0 : ROW_CHUNKS0[0], :],
            )
            nc.sync.dma_start(
                out=klhs_sb[:], in_=wk.rearrange("i t o -> i (t o)")
            )
            c0 = ROW_CHUNKS0[0]
            for r1 in ROW_CHUNKS0[1:]:
                nc.sync.dma_start(out=xps30[:, c0:r1, :], in_=xp3d0[:, c0:r1, :])
                c0 = r1
            for b in range(1, BPC):
                xp3d = xflat[b].rearrange("i (h w) -> i h w", w=WP)
                xps3 = xp_sbs[b].rearrange("i (h w) -> i h w", w=WP)
                nc.sync.dma_start(out=xps3[:], in_=xp3d[:])
            nc.sync.dma_start(out=bias_sb[:], in_=bb[:])

            for b in range(BPC):
                # Image 0 gates the whole pipeline: it arrives in row-chunks
                # (DMA'd above) and its box-sums/matmuls are emitted per
                # chunk, so the PE stream starts as soon as chunk 1 lands.
                if b == 0:
                    row_chunks = ROW_CHUNKS0
                    tile_groups = [[0], [1, 2], [3, 4], [5, 6]]
                else:
                    row_chunks = [HP]
                    tile_groups = [list(range(NT))]

                xp_sb = xp_sbs[b]

                # Distinct tags so each tensor gets its own ring: temporaries
                # (read once, immediately) single-buffered; matmul inputs
                # double-buffered so image b+1's box-sums overlap image b's
                # matmuls instead of waiting for its ring slot's last reader.
                xvt = vpool.tile([CI, NV], BF16, tag="xvt", bufs=1)
                xv = vpool.tile([CI, NV], BF16, tag="xv", bufs=2)
                xht = hpool.tile([CI, NPAD], BF16, tag="xht", bufs=1)
                xh = hpool.tile([CI, NPAD], BF16, tag="xh", bufs=2)
                ppt = ppool.tile([CI, NV], BF16, tag="ppt", bufs=1)
                pp = ppool.tile([CI, NV], BF16, tag="pp", bufs=2)

                xv3 = xv.rearrange("i (h w) -> i h w", w=WP)   # [128, 56, 58]
                xh3 = xh.rearrange("i (h w) -> i h w", w=WP)   # [128, 58, 58]
                pp3 = pp.rearrange("i (h w) -> i h w", w=WP)   # [128, 56, 58]

                youts = {}

                def rhs_for(s, t, xv3=xv3, xh3=xh3, pp3=pp3):
                    h0 = t * ROWT
                    if s == 0:
                        return xv3[:, h0 : h0 + ROWT, 0:W]
                    if s == 1:
                        return xv3[:, h0 : h0 + ROWT, 1 : 1 + W]
                    if s == 2:
                        return xh3[:, h0 : h0 + ROWT, 0:W]
                    if s == 3:
                        return xh3[:, h0 + 1 : h0 + 1 + ROWT, 0:W]
                    return pp3[:, h0 : h0 + ROWT, 0:W]

                def emit_tile(half, t, b=b, youts=youts):
                    if half not in youts:
                        youts[half] = ypool.tile(
                            [128, NPIX], BF16, name=f"yout_{b}_{half}", tag="yout"
                        )
                    yout = youts[half]
                    ps = pspool.tile(
                        [128, NTILE], F32, name=f"ps_{b}_{half}_{t}", tag="ps"
                    )
                    for s in range(NSL):
                        nc.tensor.matmul(
                            ps[:],
                            kl3[:, s, half * 128 : half * 128 + 128],
                            rhs_for(s, t),
                            start=(s == 0),
                            stop=(s == NSL - 1),
                        )
                    # drain PSUM -> SBUF (bf16, +bias), then DMA in batches
                    nc.scalar.activation(
                        yout[:, t * NTILE : (t + 1) * NTILE],
                        ps[:],
                        mybir.ActivationFunctionType.Identity,
                        bias=bias_sb[:, half : half + 1],
                    )
                    if b == BPC - 1 and half == 1 and t >= 4:
                        # final half-block: per-tile stores so the kernel
                        # tail only waits on one small DMA
                        nc.sync.dma_start(
                            out=yflat[
                                b,
                                half * 128 : half * 128 + 128,
                                t * NTILE : (t + 1) * NTILE,
                            ],
                            in_=yout[:, t * NTILE : (t + 1) * NTILE],
                        )
                    elif t == 3:
                        nc.sync.dma_start(
                            out=yflat[b, half * 128 : half * 128 + 128, 0 : 4 * NTILE],
                            in_=yout[:, 0 : 4 * NTILE],
                        )
                    elif t == NT - 1:
                        nc.sync.dma_start(
                            out=yflat[b, half * 128 : half * 128 + 128, 4 * NTILE : NPIX],
                            in_=yout[:, 4 * NTILE : NPIX],
                        )

                v0 = h0r = 0
                for ci, r1 in enumerate(row_chunks):

                    # box-sums for this chunk, then the tiles it unlocks:
                    # xv[j] = xp[j] + xp[j+58] + xp[j+116]   (rows 0..55)
                    # xh[j] = xp[j] + xp[j+1] + xp[j+2]      (rows 0..57)
                    # P[j]  = xv[j] + xv[j+1] + xv[j+2]      (rows 0..55)
                    last = ci == len(row_chunks) - 1
                    v1 = H if last else r1 - 2        # xv rows ready
                    h1 = HP if last else r1 - 1       # xh rows ready
                    a, z = v0 * WP, v1 * WP
                    nc.vector.tensor_add(
                        xvt[:, a:z], xp_sb[:, a:z], xp_sb[:, a + WP : z + WP]
                    )
                    nc.vector.tensor_add(
                        xv[:, a:z], xvt[:, a:z], xp_sb[:, a + 2 * WP : z + 2 * WP]
                    )
                    # xh chain runs on gpsimd for images 1-3 (vector alone
                    # cannot keep all four images' box-sums ahead of the PE;
                    # gpsimd at ~1/3 the rate still finishes each image's xh
                    # a block early).  Image 0 is latency-critical: vector.
                    heng = nc.vector if b == 0 else nc.gpsimd
                    a, z = h0r * WP, h1 * WP - 2
                    heng.tensor_add(
                        xht[:, a:z], xp_sb[:, a:z], xp_sb[:, a + 1 : z + 1]
                    )
                    heng.tensor_add(
                        xh[:, a:z], xht[:, a:z], xp_sb[:, a + 2 : z + 2]
                    )
                    a, z = v0 * WP, v1 * WP - 2
                    nc.vector.tensor_add(
                        ppt[:, a:z], xv[:, a:z], xv[:, a + 1 : z + 1]
                    )
                    nc.vector.tensor_add(
                        pp[:, a:z], ppt[:, a:z], xv[:, a + 2 : z + 2]
                    )
                    v0, h0r = v1, h1

                    for t in tile_groups[ci]:
                        emit_tile(0, t)
                        emit_tile(1, t)
    nc.finalize()
    return nc


_NC_CACHE = {}


def _get_nc():
    if "nc" not in _NC_CACHE:
        _NC_CACHE["nc"] = build_nc()
    return _NC_CACHE["nc"]


def make_in_maps(x, weight, bias):
    x = np.asarray(x, dtype=np.float32)
    weight = np.asarray(weight, dtype=np.float32)
    bias = np.asarray(bias, dtype=np.float32)

    krow = weight.sum(axis=3)  # [O, I, 3]
    kcol = weight.sum(axis=2)  # [O, I, 3]
    klhs = np.empty((CI, NSL, CO), np.float32)
    klhs[:, 0, :] = (krow[:, :, 0] - krow[:, :, 2]).T
    klhs[:, 1, :] = (krow[:, :, 1] - krow[:, :, 2]).T
    klhs[:, 2, :] = (kcol[:, :, 0] - kcol[:, :, 2]).T
    klhs[:, 3, :] = (kcol[:, :, 1] - kcol[:, :, 2]).T
    klhs[:, 4, :] = (krow[:, :, 2] + kcol[:, :, 2]).T
    klhs = klhs.astype(ml_dtypes.bfloat16)

    xp = np.zeros((B, CI, HP, WP), np.float32)
    xp[:, :, 1 : H + 1, 1 : W + 1] = x
    xp = xp.astype(ml_dtypes.bfloat16)

    bias2 = np.ascontiguousarray(bias.reshape(2, 128).T)  # [128, 2] f32

    return [
        {"xp": xp[c * BPC : (c + 1) * BPC], "klhs": klhs, "bias2": bias2}
        for c in range(NCORES)
    ]


def run(in_maps, **kwargs):
    nc = _get_nc()
    return run_bass_kernel_spmd(nc, in_maps, list(range(NCORES)), **kwargs)


def kernel(x, weight, bias):
    res = run(make_in_maps(x, weight, bias))
    out = np.concatenate([res.results[c]["y"] for c in range(NCORES)], axis=0)
    return out.astype(np.float32)


# revision 16
# speedup vs baseline: 1.3043x; 1.3043x over previous
"""Trainium2 Bass kernel for CommutatorConv2d.

Math: with lambda_c=0, lambda_a=1 the reference is a conv2d with effective
kernel  w_eff[o,i,r,s] = krow[o,i,s] + kcol[o,i,r]  (krow = sum_r w, kcol =
sum_s w).  The 9-tap conv factors into 1D convs over box-summed inputs, and
because sum_s xv_s == sum_r xh_r == P (the 3x3 patch sum), the 6 natural
contraction slices collapse to 5:

  y[o,h,w] = (krow0-krow2)[o,i] * xv[i, h, w]
           + (krow1-krow2)[o,i] * xv[i, h, w+1]
           + (kcol0-kcol2)[o,i] * xh[i, h,   w]
           + (kcol1-kcol2)[o,i] * xh[i, h+1, w]
           + (krow2+kcol2)[o,i] * P [i, h, w]      + bias[o]

where xv = vertical 3-tap sum of zero-padded x, xh = horizontal 3-tap sum,
P = horizontal 3-tap sum of xv.  Per output tile that is 5 accumulating
matmuls (contraction 128 each) instead of 9 for direct conv / 6 for the
two-1D-conv form.  Weight differences are computed host-side for free.

Schedule: tile-major (a tile's 5 matmuls then its PSUM drain) so drains and
output stores spread across the whole run and the kernel tail is one tile
deep.  Image 0 arrives in 4 row-chunks with box-sums and matmuls emitted
per chunk: the PE stream starts early and never gaps, which also keeps the
HAM clock-gate from re-throttling mid-run.

Output is stored bf16 on device (halves store DMA) and upcast on host.

Sharding: data-parallel over batch; 4 images per core on 8 cores.
"""

import numpy as np
import ml_dtypes

import concourse.bass as bass
import concourse.bacc as bacc
import concourse.mybir as mybir
import concourse.tile as tile
from concourse.bass_utils import run_bass_kernel_spmd

B, CI, CO, H, W = 32, 128, 256, 56, 56
NCORES = 8
BPC = B // NCORES          # images per core
HP, WP = H + 2, W + 2      # padded spatial dims
NPIX = H * W               # 3136
ROWT = 8                   # output rows per matmul tile
NT = H // ROWT             # 7 pixel tiles per image
NTILE = ROWT * W           # 448 columns per matmul
NSL = 5                    # contraction slices

F32 = mybir.dt.float32
BF16 = mybir.dt.bfloat16


def build_nc():
    nc = bacc.Bacc(None, enable_partition_id=False)
    xin = nc.declare_dram_parameter("xp", [BPC, CI, HP, WP], BF16, isOutput=False)
    wk = nc.declare_dram_parameter("klhs", [CI, NSL, CO], BF16, isOutput=False)
    bb = nc.declare_dram_parameter("bias2", [CI, 2], F32, isOutput=False)
    y = nc.declare_dram_parameter("y", [BPC, CO, H, W], BF16, isOutput=True)

    xflat = xin.rearrange("b c h w -> b c (h w)")
    yflat = y.rearrange("b o h w -> b o (h w)")
    NPAD = HP * WP           # 3364
    NV = H * WP              # 3248 (rows 0..55 of padded, all 58 cols)

    with tile.TileContext(nc) as tc:
        with (
            tc.tile_pool(name="const", bufs=1) as cpool,
            tc.tile_pool(name="xp", bufs=BPC) as xpool,
            tc.tile_pool(name="xv", bufs=2) as vpool,
            tc.tile_pool(name="xh", bufs=2) as hpool,
            tc.tile_pool(name="pp", bufs=2) as ppool,
            tc.tile_pool(name="yo", bufs=3) as ypool,
            tc.tile_pool(name="ps", bufs=7, space="PSUM") as pspool,
        ):
            klhs_sb = cpool.tile([CI, NSL * CO], BF16)
            bias_sb = cpool.tile([CI, 2], F32)
            kl3 = klhs_sb.rearrange("i (t o) -> i t o", o=CO)

            # PE warmup: a burst of matmuls on zeros bridges the PE from
            # engine-open until the first real matmul's data has landed
            # (DMA + box-sum chain, ~5us), so the HAM clock-gate reaches
            # 2.4 GHz before the real stream begins and never re-throttles.
            warm = cpool.tile([128, NTILE], BF16)
            nc.vector.memset(warm[:], 0.0)
            warm_ps = pspool.tile([128, NTILE], F32, bufs=1, tag="warm")
            for _ in range(11):
                nc.tensor.matmul(
                    warm_ps[:], warm[:, 0:128], warm[:], start=True, stop=True
                )
            warm_out = cpool.tile([128, 32], F32)
            nc.scalar.activation(
                warm_out[:], warm_ps[:, 0:32], mybir.ActivationFunctionType.Copy
            )

            # All input DMAs are issued before any compute/store is emitted:
            # the sync queue issues strictly in program order, so a load
            # emitted after a store would wait for that store's (compute-
            # gated) semaphore — serializing input prefetch behind compute.
            ROW_CHUNKS0 = [12, 30, 44, HP]
            xp_sbs = []
            for b in range(BPC):
                xp_sb = xpool.tile([CI, NPAD], BF16, name=f"xp_{b}")
                xp_sbs.append(xp_sb)
            # chunk 1 of image 0 leads (it gates the box-sum chain, which is
            # longer than the weights->first-matmul path), then weights,
            # then the rest of the input.
            xp3d0 = xflat[0].rearrange("i (h w) -> i h w", w=WP)
            xps30 = xp_sbs[0].rearrange("i (h w) -> i h w", w=WP)
            nc.sync.dma_start(
                out=xps30[:, 0 : ROW_CHUNKS0[0], :],
                in_=xp3d0[:, 0 : ROW_CHUNKS0[0], :],
            )
            nc.sync.dma_start(
                out=klhs_sb[:], in_=wk.rearrange("i t o -> i (t o)")
            )
            nc.sync.dma_start(out=bias_sb[:], in_=bb[:])
            c0 = ROW_CHUNKS0[0]
            for r1 in ROW_CHUNKS0[1:]:
                nc.sync.dma_start(out=xps30[:, c0:r1, :], in_=xp3d0[:, c0:r1, :])
                c0 = r1
            for b in range(1, BPC):
                xp3d = xflat[b].rearrange("i (h w) -> i h w", w=WP)
                xps3 = xp_sbs[b].rearrange("i (h w) -> i h w", w=WP)
                nc.sync.dma_start(out=xps3[:], in_=xp3d[:])

            for b in range(BPC):
                # Image 0 gates the whole pipeline: it arrives in row-chunks
                # (DMA'd above) and its box-sums/matmuls are emitted per
                # chunk, so the PE stream starts as soon as chunk 1 lands.
                if b == 0:
                    row_chunks = ROW_CHUNKS0
                    tile_groups = [[0], [1, 2], [3, 4], [5, 6]]
                else:
                    row_chunks = [HP]
                    tile_groups = [list(range(NT))]

                xp_sb = xp_sbs[b]

                # Distinct tags so each tensor gets its own ring: temporaries
                # (read once, immediately) single-buffered; matmul inputs
                # double-buffered so image b+1's box-sums overlap image b's
                # matmuls instead of waiting for its ring slot's last reader.
                xvt = vpool.tile([CI, NV], BF16, tag="xvt", bufs=1)
                xv = vpool.tile([CI, NV], BF16, tag="xv", bufs=2)
                xht = hpool.tile([CI, NPAD], BF16, tag="xht", bufs=1)
                xh = hpool.tile([CI, NPAD], BF16, tag="xh", bufs=2)
                ppt = ppool.tile([CI, NV], BF16, tag="ppt", bufs=1)
                pp = ppool.tile([CI, NV], BF16, tag="pp", bufs=2)

                xv3 = xv.rearrange("i (h w) -> i h w", w=WP)   # [128, 56, 58]
                xh3 = xh.rearrange("i (h w) -> i h w", w=WP)   # [128, 58, 58]
                pp3 = pp.rearrange("i (h w) -> i h w", w=WP)   # [128, 56, 58]

                youts = {}

                def rhs_for(s, t, xv3=xv3, xh3=xh3, pp3=pp3):
                    h0 = t * ROWT
                    if s == 0:
                        return xv3[:, h0 : h0 + ROWT, 0:W]
                    if s == 1:
                        return xv3[:, h0 : h0 + ROWT, 1 : 1 + W]
                    if s == 2:
                        return xh3[:, h0 : h0 + ROWT, 0:W]
                    if s == 3:
                        return xh3[:, h0 + 1 : h0 + 1 + ROWT, 0:W]
                    return pp3[:, h0 : h0 + ROWT, 0:W]

                def emit_tile(half, t, b=b, youts=youts):
                    if half not in youts:
                        youts[half] = ypool.tile(
                            [128, NPIX], BF16, name=f"yout_{b}_{half}", tag="yout"
                        )
                    yout = youts[half]
                    ps = pspool.tile(
                        [128, NTILE], F32, name=f"ps_{b}_{half}_{t}", tag="ps"
                    )
                    for s in range(NSL):
                        nc.tensor.matmul(
                            ps[:],
                            kl3[:, s, half * 128 : half * 128 + 128],
                            rhs_for(s, t),
                            start=(s == 0),
                            stop=(s == NSL - 1),
                        )
                    # drain PSUM -> SBUF (bf16, +bias), then DMA in batches
                    nc.scalar.activation(
                        yout[:, t * NTILE : (t + 1) * NTILE],
                        ps[:],
                        mybir.ActivationFunctionType.Identity,
                        bias=bias_sb[:, half : half + 1],
                    )
                    if b == BPC - 1 and half == 1 and t >= 4:
                        # final half-block: per-tile stores so the kernel
                        # tail only waits on one small DMA
                        nc.sync.dma_start(
                            out=yflat[
                                b,
                                half * 128 : half * 128 + 128,
                                t * NTILE : (t + 1) * NTILE,
                            ],
                            in_=yout[:, t * NTILE : (t + 1) * NTILE],
                        )
                    elif t == 3:
                        nc.sync.dma_start(
                            out=yflat[b, half * 128 : half * 128 + 128, 0 : 4 * NTILE],
                            in_=yout[:, 0 : 4 * NTILE],
                        )
                    elif t == NT - 1:
                        nc.sync.dma_start(
                            out=yflat[b, half * 128 : half * 128 + 128, 4 * NTILE : NPIX],
                            in_=yout[:, 4 * NTILE : NPIX],
                        )

                v0 = h0r = 0
                for ci, r1 in enumerate(row_chunks):

                    # box-sums for this chunk, then the tiles it unlocks:
                    # xv[j] = xp[j] + xp[j+58] + xp[j+116]   (rows 0..55)
                    # xh[j] = xp[j] + xp[j+1] + xp[j+2]      (rows 0..57)
                    # P[j]  = xv[j] + xv[j+1] + xv[j+2]      (rows 0..55)
                    last = ci == len(row_chunks) - 1
                    v1 = H if last else r1 - 2        # xv rows ready
                    h1 = HP if last else r1 - 1       # xh rows ready
                    a, z = v0 * WP, v1 * WP
                    nc.vector.tensor_add(
                        xvt[:, a:z], xp_sb[:, a:z], xp_sb[:, a + WP : z + WP]
                    )
                    nc.vector.tensor_add(
                        xv[:, a:z], xvt[:, a:z], xp_sb[:, a + 2 * WP : z + 2 * WP]
                    )
                    a, z = h0r * WP, h1 * WP - 2
                    nc.vector.tensor_add(
                        xht[:, a:z], xp_sb[:, a:z], xp_sb[:, a + 1 : z + 1]
                    )
                    nc.vector.tensor_add(
                        xh[:, a:z], xht[:, a:z], xp_sb[:, a + 2 : z + 2]
                    )
                    a, z = v0 * WP, v1 * WP - 2
                    nc.vector.tensor_add(
                        ppt[:, a:z], xv[:, a:z], xv[:, a + 1 : z + 1]
                    )
                    nc.vector.tensor_add(
                        pp[:, a:z], ppt[:, a:z], xv[:, a + 2 : z + 2]
                    )
                    v0, h0r = v1, h1

                    for t in tile_groups[ci]:
                        emit_tile(0, t)
                        emit_tile(1, t)
    nc.finalize()
    return nc


_NC_CACHE = {}


def _get_nc():
    if "nc" not in _NC_CACHE:
        _NC_CACHE["nc"] = build_nc()
    return _NC_CACHE["nc"]


def make_in_maps(x, weight, bias):
    x = np.asarray(x, dtype=np.float32)
    weight = np.asarray(weight, dtype=np.float32)
    bias = np.asarray(bias, dtype=np.float32)

    krow = weight.sum(axis=3)  # [O, I, 3]
    kcol = weight.sum(axis=2)  # [O, I, 3]
    klhs = np.empty((CI, NSL, CO), np.float32)
    klhs[:, 0, :] = (krow[:, :, 0] - krow[:, :, 2]).T
    klhs[:, 1, :] = (krow[:, :, 1] - krow[:, :, 2]).T
    klhs[:, 2, :] = (kcol[:, :, 0] - kcol[:, :, 2]).T
    klhs[:, 3, :] = (kcol[:, :, 1] - kcol[:, :, 2]).T
    klhs[:, 4, :] = (krow[:, :, 2] + kcol[:, :, 2]).T
    klhs = klhs.astype(ml_dtypes.bfloat16)

    xp = np.zeros((B, CI, HP, WP), np.float32)
    xp[:, :, 1 : H + 1, 1 : W + 1] = x
    xp = xp.astype(ml_dtypes.bfloat16)

    bias2 = np.ascontiguousarray(bias.reshape(2, 128).T)  # [128, 2] f32

    return [
        {"xp": xp[c * BPC : (c + 1) * BPC], "klhs": klhs, "bias2": bias2}
        for c in range(NCORES)
    ]


def run(in_maps, **kwargs):
    nc = _get_nc()
    return run_bass_kernel_spmd(nc, in_maps, list(range(NCORES)), **kwargs)


def kernel(x, weight, bias):
    res = run(make_in_maps(x, weight, bias))
    out = np.concatenate([res.results[c]["y"] for c in range(NCORES)], axis=0)
    return out.astype(np.float32)


# revision 18
# speedup vs baseline: 1.3086x; 1.0033x over previous
"""Trainium2 Bass kernel for CommutatorConv2d.

Math: with lambda_c=0, lambda_a=1 the reference is a conv2d with effective
kernel  w_eff[o,i,r,s] = krow[o,i,s] + kcol[o,i,r]  (krow = sum_r w, kcol =
sum_s w).  The 9-tap conv factors into 1D convs over box-summed inputs, and
because sum_s xv_s == sum_r xh_r == P (the 3x3 patch sum), the 6 natural
contraction slices collapse to 5:

  y[o,h,w] = (krow0-krow2)[o,i] * xv[i, h, w]
           + (krow1-krow2)[o,i] * xv[i, h, w+1]
           + (kcol0-kcol2)[o,i] * xh[i, h,   w]
           + (kcol1-kcol2)[o,i] * xh[i, h+1, w]
           + (krow2+kcol2)[o,i] * P [i, h, w]      + bias[o]

where xv = vertical 3-tap sum of zero-padded x, xh = horizontal 3-tap sum,
P = horizontal 3-tap sum of xv.  Per output tile that is 5 accumulating
matmuls (contraction 128 each) instead of 9 for direct conv / 6 for the
two-1D-conv form.  Weight differences are computed host-side for free.

Schedule: tile-major (a tile's 5 matmuls then its PSUM drain) so drains and
output stores spread across the whole run and the kernel tail is one tile
deep.  Image 0 arrives in 4 row-chunks with box-sums and matmuls emitted
per chunk: the PE stream starts early and never gaps, which also keeps the
HAM clock-gate from re-throttling mid-run.

Output is stored bf16 on device (halves store DMA) and upcast on host.

Sharding: data-parallel over batch; 4 images per core on 8 cores.
"""

import numpy as np
import ml_dtypes

import concourse.bass as bass
import concourse.bacc as bacc
import concourse.mybir as mybir
import concourse.tile as tile
from concourse.bass_utils import run_bass_kernel_spmd

B, CI, CO, H, W = 32, 128, 256, 56, 56
NCORES = 8
BPC = B // NCORES          # images per core
HP, WP = H + 2, W + 2      # padded spatial dims
NPIX = H * W               # 3136
ROWT = 8                   # output rows per matmul tile
NT = H // ROWT             # 7 pixel tiles per image
NTILE = ROWT * W           # 448 columns per matmul
NSL = 5                    # contraction slices

F32 = mybir.dt.float32
BF16 = mybir.dt.bfloat16


def build_nc():
    nc = bacc.Bacc(None, enable_partition_id=False)
    xin = nc.declare_dram_parameter("xp", [BPC, CI, HP, WP], BF16, isOutput=False)
    wk = nc.declare_dram_parameter("klhs", [CI, NSL, CO], BF16, isOutput=False)
    bb = nc.declare_dram_parameter("bias2", [CI, 2], F32, isOutput=False)
    y = nc.declare_dram_parameter("y", [BPC, CO, H, W], BF16, isOutput=True)

    xflat = xin.rearrange("b c h w -> b c (h w)")
    yflat = y.rearrange("b o h w -> b o (h w)")
    NPAD = HP * WP           # 3364
    NV = H * WP              # 3248 (rows 0..55 of padded, all 58 cols)

    with tile.TileContext(nc) as tc:
        with (
            tc.tile_pool(name="const", bufs=1) as cpool,
            tc.tile_pool(name="xp", bufs=BPC) as xpool,
            tc.tile_pool(name="xv", bufs=2) as vpool,
            tc.tile_pool(name="xh", bufs=2) as hpool,
            tc.tile_pool(name="pp", bufs=2) as ppool,
            tc.tile_pool(name="yo", bufs=3) as ypool,
            tc.tile_pool(name="ps", bufs=7, space="PSUM") as pspool,
        ):
            klhs_sb = cpool.tile([CI, NSL * CO], BF16)
            bias_sb = cpool.tile([CI, 2], F32)
            kl3 = klhs_sb.rearrange("i (t o) -> i t o", o=CO)

            # PE warmup: a burst of matmuls on zeros bridges the PE from
            # engine-open until the first real matmul's data has landed
            # (DMA + box-sum chain, ~5us), so the HAM clock-gate reaches
            # 2.4 GHz before the real stream begins and never re-throttles.
            warm = cpool.tile([128, NTILE], BF16)
            nc.vector.memset(warm[:], 0.0)
            warm_ps = pspool.tile([128, NTILE], F32, bufs=1, tag="warm")
            for _ in range(11):
                nc.tensor.matmul(
                    warm_ps[:], warm[:, 0:128], warm[:], start=True, stop=True
                )
            warm_out = cpool.tile([128, 32], F32)
            nc.scalar.activation(
                warm_out[:], warm_ps[:, 0:32], mybir.ActivationFunctionType.Copy
            )

            # All input DMAs are issued before any compute/store is emitted:
            # the sync queue issues strictly in program order, so a load
            # emitted after a store would wait for that store's (compute-
            # gated) semaphore — serializing input prefetch behind compute.
            ROW_CHUNKS0 = [12, 30, 44, HP]
            xp_sbs = []
            for b in range(BPC):
                xp_sb = xpool.tile([CI, NPAD], BF16, name=f"xp_{b}")
                xp_sbs.append(xp_sb)
            # chunk 1 of image 0 leads (it gates the box-sum chain, which is
            # longer than the weights->first-matmul path), then weights,
            # then the rest of the input.
            xp3d0 = xflat[0].rearrange("i (h w) -> i h w", w=WP)
            xps30 = xp_sbs[0].rearrange("i (h w) -> i h w", w=WP)
            nc.sync.dma_start(
                out=xps30[:, 0 : ROW_CHUNKS0[0], :],
                in_=xp3d0[:, 0 : ROW_CHUNKS0[0], :],
            )
            nc.sync.dma_start(
                out=klhs_sb[:], in_=wk.rearrange("i t o -> i (t o)")
            )
            nc.sync.dma_start(out=bias_sb[:], in_=bb[:])
            c0 = ROW_CHUNKS0[0]
            for r1 in ROW_CHUNKS0[1:]:
                nc.sync.dma_start(out=xps30[:, c0:r1, :], in_=xp3d0[:, c0:r1, :])
                c0 = r1
            for b in range(1, BPC):
                xp3d = xflat[b].rearrange("i (h w) -> i h w", w=WP)
                xps3 = xp_sbs[b].rearrange("i (h w) -> i h w", w=WP)
                nc.sync.dma_start(out=xps3[:], in_=xp3d[:])

            for b in range(BPC):
                # Image 0 gates the whole pipeline: it arrives in row-chunks
                # (DMA'd above) and its box-sums/matmuls are emitted per
                # chunk, so the PE stream starts as soon as chunk 1 lands.
                if b == 0:
                    row_chunks = ROW_CHUNKS0
                    tile_groups = [[0], [1, 2], [3, 4], [5, 6]]
                else:
                    row_chunks = [HP]
                    tile_groups = [list(range(NT))]

                xp_sb = xp_sbs[b]

                # Distinct tags so each tensor gets its own ring: temporaries
                # (read once, immediately) single-buffered; matmul inputs
                # double-buffered so image b+1's box-sums overlap image b's
                # matmuls instead of waiting for its ring slot's last reader.
                xvt = vpool.tile([CI, NV], BF16, tag="xvt", bufs=1)
                xv = vpool.tile([CI, NV], BF16, tag="xv", bufs=2)
                xht = hpool.tile([CI, NPAD], BF16, tag="xht", bufs=1)
                xh = hpool.tile([CI, NPAD], BF16, tag="xh", bufs=2)
                ppt = ppool.tile([CI, NV], BF16, tag="ppt", bufs=1)
                pp = ppool.tile([CI, NV], BF16, tag="pp", bufs=2)

                xv3 = xv.rearrange("i (h w) -> i h w", w=WP)   # [128, 56, 58]
                xh3 = xh.rearrange("i (h w) -> i h w", w=WP)   # [128, 58, 58]
                pp3 = pp.rearrange("i (h w) -> i h w", w=WP)   # [128, 56, 58]

                youts = {}

                def rhs_for(s, t, xv3=xv3, xh3=xh3, pp3=pp3):
                    h0 = t * ROWT
                    if s == 0:
                        return xv3[:, h0 : h0 + ROWT, 0:W]
                    if s == 1:
                        return xv3[:, h0 : h0 + ROWT, 1 : 1 + W]
                    if s == 2:
                        return xh3[:, h0 : h0 + ROWT, 0:W]
                    if s == 3:
                        return xh3[:, h0 + 1 : h0 + 1 + ROWT, 0:W]
                    return pp3[:, h0 : h0 + ROWT, 0:W]

                pstiles = {}

                def emit_mm(half, s, t, b=b, pstiles=pstiles):
                    if (half, t) not in pstiles:
                        pstiles[(half, t)] = pspool.tile(
                            [128, NTILE], F32, name=f"ps_{b}_{half}_{t}", tag="ps"
                        )
                    nc.tensor.matmul(
                        pstiles[(half, t)][:],
                        kl3[:, s, half * 128 : half * 128 + 128],
                        rhs_for(s, t),
                        start=(s == 0),
                        stop=(s == NSL - 1),
                    )
                    if s == NSL - 1:
                        emit_drain(half, t)

                def emit_drain(half, t, b=b, youts=youts, pstiles=pstiles):
                    if half not in youts:
                        youts[half] = ypool.tile(
                            [128, NPIX], BF16, name=f"yout_{b}_{half}", tag="yout"
                        )
                    yout = youts[half]
                    ps = pstiles[(half, t)]
                    # drain PSUM -> SBUF (bf16, +bias), then DMA in batches
                    nc.scalar.activation(
                        yout[:, t * NTILE : (t + 1) * NTILE],
                        ps[:],
                        mybir.ActivationFunctionType.Identity,
                        bias=bias_sb[:, half : half + 1],
                    )
                    if b == BPC - 1 and half == 1 and t >= 4:
                        # final half-block: per-tile stores so the kernel
                        # tail only waits on one small DMA
                        nc.sync.dma_start(
                            out=yflat[
                                b,
                                half * 128 : half * 128 + 128,
                                t * NTILE : (t + 1) * NTILE,
                            ],
                            in_=yout[:, t * NTILE : (t + 1) * NTILE],
                        )
                    elif t == 3:
                        nc.sync.dma_start(
                            out=yflat[b, half * 128 : half * 128 + 128, 0 : 4 * NTILE],
                            in_=yout[:, 0 : 4 * NTILE],
                        )
                    elif t == NT - 1:
                        nc.sync.dma_start(
                            out=yflat[b, half * 128 : half * 128 + 128, 4 * NTILE : NPIX],
                            in_=yout[:, 4 * NTILE : NPIX],
                        )

                v0 = h0r = 0
                for ci, r1 in enumerate(row_chunks):

                    # box-sums for this chunk, then the tiles it unlocks:
                    # xv[j] = xp[j] + xp[j+58] + xp[j+116]   (rows 0..55)
                    # xh[j] = xp[j] + xp[j+1] + xp[j+2]      (rows 0..57)
                    # P[j]  = xv[j] + xv[j+1] + xv[j+2]      (rows 0..55)
                    last = ci == len(row_chunks) - 1
                    v1 = H if last else r1 - 2        # xv rows ready
                    h1 = HP if last else r1 - 1       # xh rows ready
                    a, z = v0 * WP, v1 * WP
                    nc.vector.tensor_add(
                        xvt[:, a:z], xp_sb[:, a:z], xp_sb[:, a + WP : z + WP]
                    )
                    nc.vector.tensor_add(
                        xv[:, a:z], xvt[:, a:z], xp_sb[:, a + 2 * WP : z + 2 * WP]
                    )
                    a, z = h0r * WP, h1 * WP - 2
                    nc.vector.tensor_add(
                        xht[:, a:z], xp_sb[:, a:z], xp_sb[:, a + 1 : z + 1]
                    )
                    nc.vector.tensor_add(
                        xh[:, a:z], xht[:, a:z], xp_sb[:, a + 2 : z + 2]
                    )
                    a, z = v0 * WP, v1 * WP - 2
                    nc.vector.tensor_add(
                        ppt[:, a:z], xv[:, a:z], xv[:, a + 1 : z + 1]
                    )
                    nc.vector.tensor_add(
                        pp[:, a:z], ppt[:, a:z], xv[:, a + 2 : z + 2]
                    )
                    v0, h0r = v1, h1

                    if b == 0:
                        # image 0: tile-major, per chunk group — each tile
                        # runs as soon as its chunk's box-sums land
                        for t in tile_groups[ci]:
                            for half in range(2):
                                for s in range(NSL):
                                    emit_mm(half, s, t)
                    else:
                        # images 1-3: half 0 slice-major (the two A-passes
                        # give the vector engine a ~5us head start before
                        # this image's xh/P are first read), half 1
                        # tile-major (spreads drains/stores; data is old
                        # by then).
                        for s in range(NSL):
                            for t in range(NT):
                                emit_mm(0, s, t)
                        for t in range(NT):
                            for s in range(NSL):
                                emit_mm(1, s, t)
    nc.finalize()
    return nc


_NC_CACHE = {}


def _get_nc():
    if "nc" not in _NC_CACHE:
        _NC_CACHE["nc"] = build_nc()
    return _NC_CACHE["nc"]


def make_in_maps(x, weight, bias):
    x = np.asarray(x, dtype=np.float32)
    weight = np.asarray(weight, dtype=np.float32)
    bias = np.asarray(bias, dtype=np.float32)

    krow = weight.sum(axis=3)  # [O, I, 3]
    kcol = weight.sum(axis=2)  # [O, I, 3]
    klhs = np.empty((CI, NSL, CO), np.float32)
    klhs[:, 0, :] = (krow[:, :, 0] - krow[:, :, 2]).T
    klhs[:, 1, :] = (krow[:, :, 1] - krow[:, :, 2]).T
    klhs[:, 2, :] = (kcol[:, :, 0] - kcol[:, :, 2]).T
    klhs[:, 3, :] = (kcol[:, :, 1] - kcol[:, :, 2]).T
    klhs[:, 4, :] = (krow[:, :, 2] + kcol[:, :, 2]).T
    klhs = klhs.astype(ml_dtypes.bfloat16)

    xp = np.zeros((B, CI, HP, WP), np.float32)
    xp[:, :, 1 : H + 1, 1 : W + 1] = x
    xp = xp.astype(ml_dtypes.bfloat16)

    bias2 = np.ascontiguousarray(bias.reshape(2, 128).T)  # [128, 2] f32

    return [
        {"xp": xp[c * BPC : (c + 1) * BPC], "klhs": klhs, "bias2": bias2}
        for c in range(NCORES)
    ]


def run(in_maps, **kwargs):
    nc = _get_nc()
    return run_bass_kernel_spmd(nc, in_maps, list(range(NCORES)), **kwargs)


def kernel(x, weight, bias):
    res = run(make_in_maps(x, weight, bias))
    out = np.concatenate([res.results[c]["y"] for c in range(NCORES)], axis=0)
    return out.astype(np.float32)


# revision 21
# speedup vs baseline: 1.3088x; 1.0002x over previous
"""Trainium2 Bass kernel for CommutatorConv2d.

Math: with lambda_c=0, lambda_a=1 the reference is a conv2d with effective
kernel  w_eff[o,i,r,s] = krow[o,i,s] + kcol[o,i,r]  (krow = sum_r w, kcol =
sum_s w).  The 9-tap conv factors into 1D convs over box-summed inputs, and
because sum_s xv_s == sum_r xh_r == P (the 3x3 patch sum), the 6 natural
contraction slices collapse to 5:

  y[o,h,w] = (krow0-krow2)[o,i] * xv[i, h, w]
           + (krow1-krow2)[o,i] * xv[i, h, w+1]
           + (kcol0-kcol2)[o,i] * xh[i, h,   w]
           + (kcol1-kcol2)[o,i] * xh[i, h+1, w]
           + (krow2+kcol2)[o,i] * P [i, h, w]      + bias[o]

where xv = vertical 3-tap sum of zero-padded x, xh = horizontal 3-tap sum,
P = horizontal 3-tap sum of xv.  Per output tile that is 5 accumulating
matmuls (contraction 128 each) instead of 9 for direct conv / 6 for the
two-1D-conv form.  Weight differences are computed host-side for free.

Schedule: tile-major (a tile's 5 matmuls then its PSUM drain) so drains and
output stores spread across the whole run and the kernel tail is one tile
deep.  Image 0 arrives in 4 row-chunks with box-sums and matmuls emitted
per chunk: the PE stream starts early and never gaps, which also keeps the
HAM clock-gate from re-throttling mid-run.

Output is stored bf16 on device (halves store DMA) and upcast on host.

Sharding: data-parallel over batch; 4 images per core on 8 cores.
"""

import numpy as np
import ml_dtypes

import concourse.bass as bass
import concourse.bacc as bacc
import concourse.mybir as mybir
import concourse.tile as tile
from concourse.bass_utils import run_bass_kernel_spmd

B, CI, CO, H, W = 32, 128, 256, 56, 56
NCORES = 8
BPC = B // NCORES          # images per core
HP, WP = H + 2, W + 2      # padded spatial dims
NPIX = H * W               # 3136
ROWT = 8                   # output rows per matmul tile
NT = H // ROWT             # 7 pixel tiles per image
NTILE = ROWT * W           # 448 columns per matmul
NSL = 5                    # contraction slices

F32 = mybir.dt.float32
BF16 = mybir.dt.bfloat16


def build_nc():
    nc = bacc.Bacc(None, enable_partition_id=False)
    xin = nc.declare_dram_parameter("xp", [BPC, CI, HP, WP], BF16, isOutput=False)
    wk = nc.declare_dram_parameter("klhs", [CI, NSL, CO], BF16, isOutput=False)
    bb = nc.declare_dram_parameter("bias2", [CI, 2], F32, isOutput=False)
    y = nc.declare_dram_parameter("y", [BPC, CO, H, W], BF16, isOutput=True)

    xflat = xin.rearrange("b c h w -> b c (h w)")
    yflat = y.rearrange("b o h w -> b o (h w)")
    NPAD = HP * WP           # 3364
    NV = H * WP              # 3248 (rows 0..55 of padded, all 58 cols)

    with tile.TileContext(nc) as tc:
        with (
            tc.tile_pool(name="const", bufs=1) as cpool,
            tc.tile_pool(name="xp", bufs=BPC) as xpool,
            tc.tile_pool(name="xv", bufs=2) as vpool,
            tc.tile_pool(name="xh", bufs=2) as hpool,
            tc.tile_pool(name="pp", bufs=2) as ppool,
            tc.tile_pool(name="yo", bufs=3) as ypool,
            tc.tile_pool(name="ps", bufs=7, space="PSUM") as pspool,
        ):
            klhs_sb = cpool.tile([CI, NSL * CO], BF16)
            bias_sb = cpool.tile([CI, 2], F32)
            kl3 = klhs_sb.rearrange("i (t o) -> i t o", o=CO)

            # PE warmup: a burst of matmuls on zeros bridges the PE from
            # engine-open until the first real matmul's data has landed
            # (DMA + box-sum chain, ~5us), so the HAM clock-gate reaches
            # 2.4 GHz before the real stream begins and never re-throttles.
            warm = cpool.tile([128, NTILE], BF16)
            nc.vector.memset(warm[:], 0.0)
            warm_ps = pspool.tile([128, NTILE], F32, bufs=1, tag="warm")
            for _ in range(11):
                nc.tensor.matmul(
                    warm_ps[:], warm[:, 0:128], warm[:], start=True, stop=True
                )
            warm_out = cpool.tile([128, 32], F32)
            nc.scalar.activation(
                warm_out[:], warm_ps[:, 0:32], mybir.ActivationFunctionType.Copy
            )

            # All input DMAs are issued before any compute/store is emitted:
            # the sync queue issues strictly in program order, so a load
            # emitted after a store would wait for that store's (compute-
            # gated) semaphore — serializing input prefetch behind compute.
            ROW_CHUNKS0 = [12, 30, 44, HP]
            xp_sbs = []
            for b in range(BPC):
                xp_sb = xpool.tile([CI, NPAD], BF16, name=f"xp_{b}")
                xp_sbs.append(xp_sb)
            # chunk 1 of image 0 leads (it gates the box-sum chain, which is
            # longer than the weights->first-matmul path), then weights,
            # then the rest of the input.
            xp3d0 = xflat[0].rearrange("i (h w) -> i h w", w=WP)
            xps30 = xp_sbs[0].rearrange("i (h w) -> i h w", w=WP)
            nc.sync.dma_start(
                out=xps30[:, 0 : ROW_CHUNKS0[0], :],
                in_=xp3d0[:, 0 : ROW_CHUNKS0[0], :],
            )
            nc.sync.dma_start(
                out=klhs_sb[:], in_=wk.rearrange("i t o -> i (t o)")
            )
            nc.sync.dma_start(out=bias_sb[:], in_=bb[:])
            c0 = ROW_CHUNKS0[0]
            for r1 in ROW_CHUNKS0[1:]:
                nc.sync.dma_start(out=xps30[:, c0:r1, :], in_=xp3d0[:, c0:r1, :])
                c0 = r1
            for b in range(1, BPC):
                xp3d = xflat[b].rearrange("i (h w) -> i h w", w=WP)
                xps3 = xp_sbs[b].rearrange("i (h w) -> i h w", w=WP)
                nc.sync.dma_start(out=xps3[:], in_=xp3d[:])

            for b in range(BPC):
                # Image 0 gates the whole pipeline: it arrives in row-chunks
                # (DMA'd above) and its box-sums/matmuls are emitted per
                # chunk, so the PE stream starts as soon as chunk 1 lands.
                if b == 0:
                    row_chunks = ROW_CHUNKS0
                    tile_groups = [[0], [1, 2], [3, 4], [5, 6]]
                else:
                    row_chunks = [HP]
                    tile_groups = [list(range(NT))]

                xp_sb = xp_sbs[b]

                # Distinct tags so each tensor gets its own ring: temporaries
                # (read once, immediately) single-buffered; matmul inputs
                # double-buffered so image b+1's box-sums overlap image b's
                # matmuls instead of waiting for its ring slot's last reader.
                xvt = vpool.tile([CI, NV], BF16, tag="xvt", bufs=1)
                xv = vpool.tile([CI, NV], BF16, tag="xv", bufs=2)
                xht = hpool.tile([CI, NPAD], BF16, tag="xht", bufs=1)
                xh = hpool.tile([CI, NPAD], BF16, tag="xh", bufs=2)
                ppt = ppool.tile([CI, NV], BF16, tag="ppt", bufs=1)
                pp = ppool.tile([CI, NV], BF16, tag="pp", bufs=2)

                xv3 = xv.rearrange("i (h w) -> i h w", w=WP)   # [128, 56, 58]
                xh3 = xh.rearrange("i (h w) -> i h w", w=WP)   # [128, 58, 58]
                pp3 = pp.rearrange("i (h w) -> i h w", w=WP)   # [128, 56, 58]

                youts = {}

                def rhs_for(s, t, xv3=xv3, xh3=xh3, pp3=pp3):
                    h0 = t * ROWT
                    if s == 0:
                        return xv3[:, h0 : h0 + ROWT, 0:W]
                    if s == 1:
                        return xv3[:, h0 : h0 + ROWT, 1 : 1 + W]
                    if s == 2:
                        return xh3[:, h0 : h0 + ROWT, 0:W]
                    if s == 3:
                        return xh3[:, h0 + 1 : h0 + 1 + ROWT, 0:W]
                    return pp3[:, h0 : h0 + ROWT, 0:W]

                pstiles = {}

                def emit_mm(half, s, t, b=b, pstiles=pstiles):
                    if (half, t) not in pstiles:
                        pstiles[(half, t)] = pspool.tile(
                            [128, NTILE], F32, name=f"ps_{b}_{half}_{t}", tag="ps"
                        )
                    nc.tensor.matmul(
                        pstiles[(half, t)][:],
                        kl3[:, s, half * 128 : half * 128 + 128],
                        rhs_for(s, t),
                        start=(s == 0),
                        stop=(s == NSL - 1),
                    )
                    if s == NSL - 1:
                        emit_drain(half, t)

                def emit_drain(half, t, b=b, youts=youts, pstiles=pstiles):
                    if half not in youts:
                        youts[half] = ypool.tile(
                            [128, NPIX], BF16, name=f"yout_{b}_{half}", tag="yout"
                        )
                    yout = youts[half]
                    ps = pstiles[(half, t)]
                    # drain PSUM -> SBUF (bf16, +bias), then DMA in batches
                    nc.scalar.activation(
                        yout[:, t * NTILE : (t + 1) * NTILE],
                        ps[:],
                        mybir.ActivationFunctionType.Identity,
                        bias=bias_sb[:, half : half + 1],
                    )
                    if b == BPC - 1 and half == 1 and t >= 4:
                        # final half-block: per-tile stores so the kernel
                        # tail only waits on one small DMA
                        nc.sync.dma_start(
                            out=yflat[
                                b,
                                half * 128 : half * 128 + 128,
                                t * NTILE : (t + 1) * NTILE,
                            ],
                            in_=yout[:, t * NTILE : (t + 1) * NTILE],
                        )
                    elif t == 3:
                        nc.sync.dma_start(
                            out=yflat[b, half * 128 : half * 128 + 128, 0 : 4 * NTILE],
                            in_=yout[:, 0 : 4 * NTILE],
                        )
                    elif t == NT - 1:
                        nc.sync.dma_start(
                            out=yflat[b, half * 128 : half * 128 + 128, 4 * NTILE : NPIX],
                            in_=yout[:, 4 * NTILE : NPIX],
                        )

                v0 = h0r = 0
                for ci, r1 in enumerate(row_chunks):

                    # box-sums for this chunk, then the tiles it unlocks:
                    # xv[j] = xp[j] + xp[j+58] + xp[j+116]   (rows 0..55)
                    # xh[j] = xp[j] + xp[j+1] + xp[j+2]      (rows 0..57)
                    # P[j]  = xv[j] + xv[j+1] + xv[j+2]      (rows 0..55)
                    last = ci == len(row_chunks) - 1
                    v1 = H if last else r1 - 2        # xv rows ready
                    h1 = HP if last else r1 - 1       # xh rows ready
                    a, z = v0 * WP, v1 * WP
                    nc.vector.tensor_add(
                        xvt[:, a:z], xp_sb[:, a:z], xp_sb[:, a + WP : z + WP]
                    )
                    nc.vector.tensor_add(
                        xv[:, a:z], xvt[:, a:z], xp_sb[:, a + 2 * WP : z + 2 * WP]
                    )
                    a, z = h0r * WP, h1 * WP - 2
                    nc.vector.tensor_add(
                        xht[:, a:z], xp_sb[:, a:z], xp_sb[:, a + 1 : z + 1]
                    )
                    nc.vector.tensor_add(
                        xh[:, a:z], xht[:, a:z], xp_sb[:, a + 2 : z + 2]
                    )
                    a, z = v0 * WP, v1 * WP - 2
                    nc.vector.tensor_add(
                        ppt[:, a:z], xv[:, a:z], xv[:, a + 1 : z + 1]
                    )
                    nc.vector.tensor_add(
                        pp[:, a:z], ppt[:, a:z], xv[:, a + 2 : z + 2]
                    )
                    v0, h0r = v1, h1

                    if b == 0:
                        # image 0: tile-major, per chunk group — each tile
                        # runs as soon as its chunk's box-sums land
                        for t in tile_groups[ci]:
                            for half in range(2):
                                for s in range(NSL):
                                    emit_mm(half, s, t)
                    else:
                        # images 1-3: half 0 slice-major (the two A-passes
                        # give the vector engine a ~5us head start before
                        # this image's xh/P are first read), half 1
                        # tile-major (spreads drains/stores; data is old
                        # by then).
                        for s in range(NSL):
                            for t in range(NT):
                                emit_mm(0, s, t)
                        for t in range(NT):
                            for s in range(NSL):
                                emit_mm(1, s, t)
    nc.finalize()
    return nc


_NC_CACHE = {}


def _get_nc():
    if "nc" not in _NC_CACHE:
        _NC_CACHE["nc"] = build_nc()
    return _NC_CACHE["nc"]


def make_in_maps(x, weight, bias):
    x = np.asarray(x, dtype=np.float32)
    weight = np.asarray(weight, dtype=np.float32)
    bias = np.asarray(bias, dtype=np.float32)

    krow = weight.sum(axis=3)  # [O, I, 3]
    kcol = weight.sum(axis=2)  # [O, I, 3]
    klhs = np.empty((CI, NSL, CO), np.float32)
    klhs[:, 0, :] = (krow[:, :, 0] - krow[:, :, 2]).T
    klhs[:, 1, :] = (krow[:, :, 1] - krow[:, :, 2]).T
    klhs[:, 2, :] = (kcol[:, :, 0] - kcol[:, :, 2]).T
    klhs[:, 3, :] = (kcol[:, :, 1] - kcol[:, :, 2]).T
    klhs[:, 4, :] = (krow[:, :, 2] + kcol[:, :, 2]).T
    klhs = klhs.astype(ml_dtypes.bfloat16)

    xp = np.zeros((B, CI, HP, WP), np.float32)
    xp[:, :, 1 : H + 1, 1 : W + 1] = x
    xp = xp.astype(ml_dtypes.bfloat16)

    bias2 = np.ascontiguousarray(bias.reshape(2, 128).T)  # [128, 2] f32

    return [
        {"xp": xp[c * BPC : (c + 1) * BPC], "klhs": klhs, "bias2": bias2}
        for c in range(NCORES)
    ]


def run(in_maps, **kwargs):
    nc = _get_nc()
    return run_bass_kernel_spmd(nc, in_maps, list(range(NCORES)), **kwargs)


def kernel(x, weight, bias):
    res = run(make_in_maps(x, weight, bias))
    out = np.concatenate([res.results[c]["y"] for c in range(NCORES)], axis=0)
    return out.astype(np.float32)
